# revision 1
# baseline (speedup 1.0000x reference)
"""Trainium2 Bass kernel for nn_NaturalCubic (natural cubic spline per (batch, channel)).

Math: reference computes, per batch b and "channel" c (c = flat_index mod 3 of
raw.reshape(B, M, C) -- a plain memory reshape of (B, C, H, W)):

    out = sum_k alpha_k * K1(xs_k, x) + a10 + a11 * x
    K1(xc, x) = xc*x*ms - 0.5*(xc+x)*ms^2 + ms^3/3,   ms = min(xc, x)
              = 0.5*xc*x*ms - ms^3/6
identity:  K1(xc, x) = 0.5*xc^2*x - xc^3/6 + relu(xc - x)^3/6      (exact, all x)

So with host-folded constants (per b, c):
    D1 = a11 + 0.5*sum_k alpha_k*xs_k^2
    D0 = a10 - (1/6)*sum_k alpha_k*xs_k^3
    w_k = alpha_k/6
    out(x) = D0 + D1*x + sum_k w_k * relu(xs_k - x)^3

Device per knot: ScalarE Square/Relu (bias=xs per-partition const, scale=-1) and
VectorE bf16 multiplies + scalar_tensor_tensor accumulate. Knots with
xs_k <= min(x) over the slice contribute exactly 0 and are pruned host-side
(instruction count padded to the max across cores -- SPMD shares one program).
"""

import sys

sys.path.insert(0, "/opt/trn_rl_repo")

from contextlib import ExitStack

import numpy as np

import concourse.bacc as bacc
import concourse.mybir as mybir
import concourse.tile as tile
from concourse.bass_utils import run_bass_kernel_spmd

# Problem constants (hardcoded per contract)
KNOTS = 10
C = 3
B, H, W = 16, 448, 448
M = H * W                 # 200704
FLAT = C * M              # 602112
P = 128
FREE = FLAT // P          # 4704 (multiple of 3 -> channel = column mod 3)
CV = FREE // C            # 1568 per-channel strided view length
N_CORES = 8
BPC = B // N_CORES        # 2 batches per core

SLOTS = BPC * C           # 6 (b_local, c) groups per core
# const columns per slot: [D0, D1, xs_0..xs_{A-1}, w_0..w_{A-1}] padded to KNOTS
SLOTW = 2 + 2 * KNOTS     # 22
NCONST = SLOTS * SLOTW    # 132

dt = mybir.dt
AF = mybir.ActivationFunctionType
OP = mybir.AluOpType

_prog_cache: dict = {}


def _build_program(counts, n_dve):
    """counts: tuple of SLOTS ints = knots per slot (max across cores).
    n_dve[s]: how many of slot s's knots run the DVE-heavy variant."""
    nc = bacc.Bacc(
        "TRN2", target_bir_lowering=False, debug=False, enable_asserts=False
    )
    x_d = nc.dram_tensor("x", (BPC, P, FREE), dt.float32, kind="ExternalInput").ap()
    c_d = nc.dram_tensor("consts", (P, NCONST), dt.float32, kind="ExternalInput").ap()
    y_d = nc.dram_tensor("y", (BPC, P, FREE), dt.float32, kind="ExternalOutput").ap()

    with ExitStack() as ctx:
        tc = ctx.enter_context(tile.TileContext(nc))
        cpool = ctx.enter_context(tc.tile_pool(name="cpool", bufs=1))
        xpool = ctx.enter_context(tc.tile_pool(name="xpool", bufs=2))
        ypool = ctx.enter_context(tc.tile_pool(name="ypool", bufs=2))
        apool = ctx.enter_context(tc.tile_pool(name="apool", bufs=3))
        tpool = ctx.enter_context(tc.tile_pool(name="tpool", bufs=6))

        ct = cpool.tile([P, NCONST], dt.float32)
        nc.sync.dma_start(out=ct[:], in_=c_d[:])

        for b in range(BPC):
            xt = xpool.tile([P, FREE], dt.float32, tag="x")
            nc.sync.dma_start(out=xt[:], in_=x_d[b])
            yt = ypool.tile([P, FREE], dt.float32, tag="y")
            for c in range(C):
                s = b * C + c
                base = s * SLOTW
                col = lambda j: ct[:, base + j : base + j + 1]
                xv = xt[:, c::C]  # (128, 1568) strided view, one spline-channel

                lin = apool.tile([P, CV], dt.float32, tag="lin")
                nc.scalar.activation(lin[:], xv, AF.Identity, bias=col(0), scale=col(1))

                acc = apool.tile([P, CV], dt.bfloat16, tag="acc")
                A = counts[s]
                for k in range(A):
                    xs_ap = col(2 + k)
                    w_ap = col(2 + KNOTS + k)
                    r = tpool.tile([P, CV], dt.bfloat16, tag="r")
                    p = tpool.tile([P, CV], dt.bfloat16, tag="p")
                    nc.scalar.activation(r[:], xv, AF.Relu, bias=xs_ap, scale=-1.0)
                    if k < n_dve[s]:
                        # DVE-heavy: cube on VectorE (bf16 2x mode)
                        s2 = tpool.tile([P, CV], dt.bfloat16, tag="s2")
                        nc.vector.tensor_tensor(s2[:], r[:], r[:], op=OP.mult)
                        nc.vector.tensor_tensor(p[:], s2[:], r[:], op=OP.mult)
                    else:
                        # ACT-heavy: square on ScalarE
                        s2 = tpool.tile([P, CV], dt.bfloat16, tag="s2")
                        nc.scalar.activation(s2[:], xv, AF.Square, bias=xs_ap, scale=-1.0)
                        nc.vector.tensor_tensor(p[:], s2[:], r[:], op=OP.mult)
                    if k == 0:
                        nc.vector.tensor_scalar(
                            acc[:], p[:], w_ap, None, op0=OP.mult
                        )
                    else:
                        nc.vector.scalar_tensor_tensor(
                            acc[:], p[:], w_ap, acc[:], op0=OP.mult, op1=OP.add
                        )
                if A > 0:
                    nc.vector.tensor_tensor(yt[:, c::C], lin[:], acc[:], op=OP.add)
                else:
                    nc.vector.tensor_copy(yt[:, c::C], lin[:])
            nc.sync.dma_start(out=y_d[b], in_=yt[:])

    nc.compile()
    return nc


def _prepare(raw, params_tensor):
    """Host-side: fold params, prune dead knots, build per-core inputs."""
    raw = np.ascontiguousarray(raw, dtype=np.float32)
    pt = np.asarray(params_tensor, dtype=np.float32)

    xs = pt[:, : C * KNOTS].reshape(B, KNOTS, C).astype(np.float64)     # (B,K,C)
    al = pt[:, C * KNOTS :].reshape(B, KNOTS + 2, C).astype(np.float64)  # (B,K+2,C)
    alpha = al[:, :KNOTS, :]
    a10, a11 = al[:, KNOTS, :], al[:, KNOTS + 1, :]
    D1 = a11 + 0.5 * np.sum(alpha * xs**2, axis=1)   # (B,C)
    D0 = a10 - np.sum(alpha * xs**3, axis=1) / 6.0   # (B,C)
    wk = alpha / 6.0                                  # (B,K,C)

    flat = raw.reshape(B, FLAT)
    # per (b, c) slice minimum (channel = flat index mod 3)
    mins = flat.reshape(B, M, C).min(axis=1)          # (B,C)

    # active knots: contribution bound |w|*relu(xs - min_x)^3 above fp32 noise
    active = [[[] for _ in range(C)] for _ in range(B)]
    for b in range(B):
        for c in range(C):
            for k in range(KNOTS):
                bound = abs(wk[b, k, c]) * max(0.0, xs[b, k, c] - mins[b, c]) ** 3
                if bound > 1e-12:
                    active[b][c].append(k)

    # per-program-slot counts = max across cores (SPMD: one shared program)
    counts = []
    for s in range(SLOTS):
        b_local, c = divmod(s, C)
        counts.append(
            max(len(active[core * BPC + b_local][c]) for core in range(N_CORES))
        )
    counts = tuple(counts)

    in_maps = []
    for core in range(N_CORES):
        consts = np.zeros((P, NCONST), dtype=np.float32)
        xbuf = np.empty((BPC, P, FREE), dtype=np.float32)
        for b_local in range(BPC):
            b = core * BPC + b_local
            xbuf[b_local] = flat[b].reshape(P, FREE)
            for c in range(C):
                s = b_local * C + c
                base = s * SLOTW
                consts[:, base + 0] = D0[b, c]
                consts[:, base + 1] = D1[b, c]
                for j, k in enumerate(active[b][c]):
                    consts[:, base + 2 + j] = xs[b, k, c]
                    consts[:, base + 2 + KNOTS + j] = wk[b, k, c]
                # padding stays zero: relu(0 - x) == 0 for x >= 0, and w == 0
        in_maps.append({"x": xbuf, "consts": consts})
    return counts, in_maps


def _get_program(counts):
    key = counts
    if key not in _prog_cache:
        n_dve = tuple(int(round(0.49 * a)) for a in counts)
        _prog_cache[key] = _build_program(counts, n_dve)
    return _prog_cache[key]


def kernel(raw, params_tensor, _trace=False, _trace_kwargs=None):
    counts, in_maps = _prepare(raw, params_tensor)
    nc = _get_program(counts)
    res = run_bass_kernel_spmd(
        nc,
        in_maps,
        list(range(N_CORES)),
        trace=_trace,
        **(_trace_kwargs or {}),
    )
    out = np.empty((B, C, H, W), dtype=np.float32)
    for core in range(N_CORES):
        y = res.results[core]["y"]  # (BPC, P, FREE)
        for b_local in range(BPC):
            b = core * BPC + b_local
            out[b] = y[b_local].reshape(C, H, W)
    kernel._last_results = res
    return out
